# revision 1
# baseline (speedup 1.0000x reference)
"""ConservationConstrainedAttention Trainium2 kernel (8 NeuronCores).

Sharding: core c -> (batch b = c//2, head-half g = c%2, 8 of 16 heads).
Per core, attention runs in transposed layout sT[k, q] so that softmax
denominators, the energy matvec and the attended matmul are all PE
contractions over the partition (k) axis; row sums are ones-matmuls whose
[128,S] outputs double as partition broadcasts.  The conservation
correction needs a global sum of attended energy (one tiny AllReduce).
Scores are recomputed in pass 2 instead of spilling 32MB of attention
weights.  Output projection partials are summed across each batch pair
with a ReduceScatter; each core layernorms its own half of the rows.
"""
import sys

sys.path.insert(0, "/opt/trn_rl_repo")

import numpy as np
import concourse.bass as bass
import concourse.tile as tile
from concourse import bacc, mybir
from concourse.bass_utils import run_bass_kernel_spmd

B, S, D, H = 4, 1024, 1024, 16
DH = D // H          # 64
NC = 8               # cores
HL = 8               # local heads per core
SH = HL * DH         # 512 head-feature slice per core
KC = S // 128        # 8 k chunks
QH = S // 2          # 512 rows of output per core

f32 = mybir.dt.float32
f32r = mybir.dt.float32r
bf16 = mybir.dt.bfloat16
AX = mybir.AxisListType
ALU = mybir.AluOpType
AF = mybir.ActivationFunctionType


def _bc(ap, pattern, off_elems=0):
    return bass.AP(tensor=ap.tensor, offset=ap.offset + off_elems, ap=pattern)


def build_kernel():
    nc = bacc.Bacc("TRN2", target_bir_lowering=False, debug=False, num_devices=NC)

    QT_D = nc.dram_tensor("queryT", [D, S], f32r, kind="ExternalInput")
    KT_D = nc.dram_tensor("keyT", [D, S], f32r, kind="ExternalInput")
    VT_D = nc.dram_tensor("valueT", [D, S], f32r, kind="ExternalInput")
    QHALF = nc.dram_tensor("query_half", [QH, D], f32, kind="ExternalInput")
    WQT = nc.dram_tensor("WqT", [D, SH], f32r, kind="ExternalInput")
    WKT = nc.dram_tensor("WkT", [D, SH], f32r, kind="ExternalInput")
    WVT = nc.dram_tensor("WvT", [D, SH], f32r, kind="ExternalInput")
    WOT = nc.dram_tensor("WoT_own", [SH, D], f32r, kind="ExternalInput")
    BQ = nc.dram_tensor("bq_half", [SH], f32, kind="ExternalInput")
    BK = nc.dram_tensor("bk_half", [SH], f32, kind="ExternalInput")
    BV = nc.dram_tensor("bv_half", [SH], f32, kind="ExternalInput")
    BO = nc.dram_tensor("bo", [D], f32, kind="ExternalInput")
    GAM = nc.dram_tensor("gamma", [D], f32, kind="ExternalInput")
    BET = nc.dram_tensor("beta", [D], f32, kind="ExternalInput")
    CO = nc.dram_tensor("coords", [S, 4], f32, kind="ExternalInput")
    CT = nc.dram_tensor("coordsT", [4, S], f32, kind="ExternalInput")
    WE = nc.dram_tensor("We_row", [1, DH], f32, kind="ExternalInput")
    BE = nc.dram_tensor("be", [1], f32, kind="ExternalInput")
    E4 = nc.dram_tensor("E4", [1, B], f32, kind="ExternalInput")
    SEL4 = nc.dram_tensor("sel4", [1, B], f32, kind="ExternalInput")
    SLOT8 = nc.dram_tensor("slot8", [1, NC], f32, kind="ExternalInput")

    OUT = nc.dram_tensor("out_half", [QH, D], f32, kind="ExternalOutput")

    scale = 1.0 / np.sqrt(DH)

    with tile.TileContext(nc) as tc:
        with (
            tc.tile_pool(name="const", bufs=1) as const,
            tc.tile_pool(name="persist", bufs=1) as persist,
            tc.tile_pool(name="work", bufs=2) as work,
            tc.tile_pool(name="u", bufs=2) as upool,
            tc.tile_pool(name="small", bufs=2) as small,
            tc.tile_pool(name="dram", bufs=1, space="DRAM") as dram,
        ):
            # ---------------- constants ----------------
            ones_f = const.tile([128, 128], f32, tag="ones_f")
            nc.vector.memset(ones_f[:], 1.0)
            ones = const.tile([128, 128], f32r, tag="ones")
            nc.vector.tensor_copy(ones[:], ones_f[:])
            eps_t = const.tile([128, 1], f32, tag="eps")
            nc.vector.memset(eps_t[:], 1e-5)
            WeB = const.tile([128, SH], f32, tag="WeB")
            nc.gpsimd.dma_start(WeB[:], _bc(WE[:], [[0, 128], [0, HL], [1, DH]]))
            beB = const.tile([128, 1], f32, tag="beB")
            nc.gpsimd.dma_start(beB[:], _bc(BE[:], [[0, 128], [1, 1]]))
            bvB = const.tile([128, SH], f32, tag="bvB")
            nc.gpsimd.dma_start(bvB[:], _bc(BV[:], [[0, 128], [1, SH]]))
            bq_c = const.tile([128, 4], f32, tag="bq_c")
            nc.sync.dma_start(bq_c[:], BQ[:].rearrange("(m p) -> p m", p=128))
            bk_c = const.tile([128, 4], f32, tag="bk_c")
            nc.sync.dma_start(bk_c[:], BK[:].rearrange("(m p) -> p m", p=128))
            e4B = const.tile([128, B], f32, tag="e4B")
            nc.gpsimd.dma_start(e4B[:], _bc(E4[:], [[0, 128], [1, B]]))
            sel4B = const.tile([128, B], f32, tag="sel4B")
            nc.gpsimd.dma_start(sel4B[:], _bc(SEL4[:], [[0, 128], [1, B]]))
            slot8_t = const.tile([1, NC], f32, tag="slot8")
            nc.sync.dma_start(slot8_t[:], SLOT8[:])

            # ---------------- persistent arrays ----------------
            QTs = [persist.tile([128, S], f32r, tag=f"QT{j}", name=f"QT{j}")
                   for j in range(4)]
            KTs = [persist.tile([128, S], f32r, tag=f"KT{j}", name=f"KT{j}")
                   for j in range(4)]
            Vs = [persist.tile([128, SH], f32r, tag=f"V{c}", name=f"V{c}")
                  for c in range(KC)]
            Ps = [persist.tile([128, S], bf16, tag=f"P{c}", name=f"P{c}")
                  for c in range(KC)]
            es = [persist.tile([128, HL], f32r, tag=f"e{c}", name=f"e{c}")
                  for c in range(KC)]
            ATs = [persist.tile([128, S], f32r, tag=f"AT{j}", name=f"AT{j}")
                   for j in range(4)]
            RRD = dram.tile([HL, S], f32, tag="rrd")

            # ---------------- stage A1: physics mask ----------------
            with tc.tile_pool(name="bcast", bufs=1) as bcast:
                cb = []
                for i in range(4):
                    t = bcast.tile([128, S], f32, tag=f"cb{i}", name=f"cb{i}")
                    nc.gpsimd.dma_start(
                        t[:], _bc(CT[:], [[0, 128], [1, S]], off_elems=i * S)
                    )
                    cb.append(t)
                for c in range(KC):
                    co = small.tile([128, 4], f32, tag="co")
                    nc.sync.dma_start(co[:], CO[c * 128:(c + 1) * 128, :])
                    dt = work.tile([128, S], f32, tag="w0")
                    nc.vector.tensor_scalar(
                        out=dt[:], in0=cb[0][:], scalar1=co[:, 0:1], scalar2=None,
                        op0=ALU.subtract,
                    )
                    dt2 = work.tile([128, S], f32, tag="w1")
                    nc.scalar.square(dt2[:], dt[:])
                    dr2 = work.tile([128, S], f32, tag="w2")
                    sq = work.tile([128, S], f32, tag="w3")
                    for i in (1, 2, 3):
                        dd = work.tile([128, S], f32, tag="w4")
                        nc.vector.tensor_scalar(
                            out=dd[:], in0=cb[i][:], scalar1=co[:, i:i + 1],
                            scalar2=None, op0=ALU.subtract,
                        )
                        if i == 1:
                            nc.scalar.square(dr2[:], dd[:])
                        else:
                            nc.scalar.square(sq[:], dd[:])
                            nc.vector.tensor_add(dr2[:], dr2[:], sq[:])
                    intv = work.tile([128, S], f32, tag="w3")
                    nc.vector.tensor_sub(intv[:], dt2[:], dr2[:])
                    absi = work.tile([128, S], f32, tag="w4")
                    nc.scalar.activation(absi[:], intv[:], AF.Abs)
                    expt = work.tile([128, S], f32, tag="w3")
                    nc.scalar.activation(expt[:], absi[:], AF.Exp, scale=-0.1)
                    nc.vector.tensor_scalar_add(expt[:], expt[:], 1e-8)
                    # not-causal = (dt2 >= dr2 or dr2 <= 1e-6) and dt >= 0
                    ge1 = work.tile([128, S], f32, tag="w4")
                    nc.vector.tensor_tensor(
                        out=ge1[:], in0=dt2[:], in1=dr2[:], op=ALU.is_ge
                    )
                    le = work.tile([128, S], f32, tag="w1")
                    nc.vector.tensor_scalar(
                        out=le[:], in0=dr2[:], scalar1=1e-6, scalar2=None,
                        op0=ALU.is_le,
                    )
                    nc.vector.tensor_tensor(
                        out=ge1[:], in0=ge1[:], in1=le[:], op=ALU.max
                    )
                    ge2 = work.tile([128, S], f32, tag="w2")
                    nc.vector.tensor_scalar(
                        out=ge2[:], in0=dt[:], scalar1=0.0, scalar2=None,
                        op0=ALU.is_ge,
                    )
                    nc.vector.tensor_mul(ge1[:], ge1[:], ge2[:])
                    nc.vector.tensor_mul(Ps[c][:], ge1[:], expt[:])

            # ---------------- stage A2: Q/K projections ----------------
            with (
                tc.tile_pool(name="xT", bufs=2) as xTp,
                tc.tile_pool(name="wT", bufs=2) as wTp,
                tc.tile_pool(name="ps_p", bufs=1, space="PSUM") as psp,
            ):
                for (XD, WD, bcol, dst) in (
                    (QT_D, WQT, bq_c, QTs),
                    (KT_D, WKT, bk_c, KTs),
                ):
                    pps = [psp.tile([128, S], f32, tag=f"pp{mc}", name=f"pp{mc}")
                           for mc in range(4)]
                    for dc in range(KC):
                        xt = xTp.tile([128, S], f32r, tag="x")
                        nc.sync.dma_start(xt[:], XD[dc * 128:(dc + 1) * 128, :])
                        wt = wTp.tile([128, SH], f32r, tag="w")
                        nc.sync.dma_start(wt[:], WD[dc * 128:(dc + 1) * 128, :])
                        for mc in range(4):
                            for nh in range(2):
                                nc.tensor.matmul(
                                    pps[mc][:, nh * 512:(nh + 1) * 512],
                                    wt[:, mc * 128:(mc + 1) * 128],
                                    xt[:, nh * 512:(nh + 1) * 512],
                                    start=(dc == 0), stop=(dc == KC - 1),
                                )
                    for mc in range(4):
                        nc.vector.tensor_scalar(
                            out=dst[mc][:], in0=pps[mc][:],
                            scalar1=bcol[:, mc:mc + 1], scalar2=None, op0=ALU.add,
                        )

            # ---------------- stage A3: V projection + energies ----------------
            with (
                tc.tile_pool(name="xT2", bufs=2) as xTp,
                tc.tile_pool(name="wT2", bufs=2) as wTp,
                tc.tile_pool(name="ps_v", bufs=1, space="PSUM") as psv,
            ):
                pvs = [psv.tile([128, SH], f32, tag=f"pv{kc}", name=f"pv{kc}")
                       for kc in range(KC)]
                for dc in range(KC):
                    xt = xTp.tile([128, S], f32r, tag="x")
                    nc.sync.dma_start(xt[:], VT_D[dc * 128:(dc + 1) * 128, :])
                    wt = wTp.tile([128, SH], f32r, tag="w")
                    nc.sync.dma_start(wt[:], WVT[dc * 128:(dc + 1) * 128, :])
                    for kc in range(KC):
                        nc.tensor.matmul(
                            pvs[kc][:],
                            xt[:, kc * 128:(kc + 1) * 128],
                            wt[:],
                            start=(dc == 0), stop=(dc == KC - 1),
                        )
                for kc in range(KC):
                    nc.vector.tensor_tensor(
                        out=Vs[kc][:], in0=pvs[kc][:], in1=bvB[:], op=ALU.add
                    )
                    tmp = work.tile([128, SH], f32, tag="w0")
                    nc.vector.tensor_tensor(
                        out=tmp[:], in0=Vs[kc][:].bitcast(f32), in1=WeB[:],
                        op=ALU.mult,
                    )
                    er = work.tile([128, HL], f32, tag="w5")
                    nc.vector.reduce_sum(
                        out=er[:],
                        in_=tmp[:].rearrange("p (h d) -> p h d", h=HL),
                        axis=AX.X,
                    )
                    nc.vector.tensor_scalar(
                        out=es[kc][:], in0=er[:], scalar1=beB[:, 0:1],
                        scalar2=None, op0=ALU.add,
                    )

            # ---------------- stage B: pass 1 softmax + energy ----------------
            part8 = small.tile([1, HL], f32, tag="part8")
            with (
                tc.tile_pool(name="ps_s", bufs=2, space="PSUM") as pss,
                tc.tile_pool(name="ps_r", bufs=1, space="PSUM") as psr,
                tc.tile_pool(name="ps_t", bufs=1, space="PSUM") as pst,
            ):
                for h in range(HL):
                    j, half = h // 2, h % 2
                    dlo = half * DH
                    pr = psr.tile([128, S], f32, tag="pr")
                    pt = pst.tile([1, S], f32, tag="pt")
                    for kc in range(KC):
                        ps_ = pss.tile([128, S], f32, tag="ps")
                        for nh in range(2):
                            nc.tensor.matmul(
                                ps_[:, nh * 512:(nh + 1) * 512],
                                KTs[j][dlo:dlo + DH, kc * 128:(kc + 1) * 128],
                                QTs[j][dlo:dlo + DH, nh * 512:(nh + 1) * 512],
                                start=True, stop=True,
                            )
                        ex = work.tile([128, S], f32, tag="w0")
                        nc.scalar.activation(ex[:], ps_[:], AF.Exp, scale=scale)
                        uc = upool.tile([128, S], f32r, tag="u")
                        nc.vector.tensor_tensor(
                            out=uc[:], in0=ex[:], in1=Ps[kc][:], op=ALU.mult
                        )
                        for nh in range(2):
                            sl = slice(nh * 512, (nh + 1) * 512)
                            nc.tensor.matmul(
                                pr[:, sl], ones[:], uc[:, sl],
                                start=(kc == 0), stop=(kc == KC - 1),
                            )
                            nc.tensor.matmul(
                                pt[:, sl], es[kc][:, h:h + 1], uc[:, sl],
                                start=(kc == 0), stop=(kc == KC - 1),
                            )
                    prs = work.tile([128, S], f32, tag="w2")
                    nc.scalar.copy(prs[:], pr[:])
                    rr = work.tile([128, S], f32, tag="w1")
                    nc.vector.reciprocal_approx_fast(out=rr[:], in_=prs[:])
                    nc.sync.dma_start(RRD[h:h + 1, :], rr[0:1, :])
                    scr = small.tile([1, S], f32, tag="scr")
                    nc.vector.tensor_tensor(out=scr[:], in0=pt[:], in1=rr[0:1, :], op=ALU.mult)
                    nc.vector.reduce_sum(out=part8[0:1, h:h + 1], in_=scr[:], axis=AX.X)

            # ---------------- stage C: global energy reduction ----------------
            ar_in = dram.tile([1, NC], f32, tag="ar_in")
            ar_out = dram.tile([1, NC], f32, tag="ar_out")
            partial = small.tile([1, 1], f32, tag="partial")
            nc.vector.reduce_sum(out=partial[:], in_=part8[:], axis=AX.X)
            ar_sb = small.tile([1, NC], f32, tag="ar_sb")
            nc.vector.tensor_scalar(
                out=ar_sb[:], in0=slot8_t[:], scalar1=partial[:, 0:1],
                scalar2=None, op0=ALU.mult,
            )
            nc.sync.dma_start(ar_in[:], ar_sb[:])
            nc.gpsimd.collective_compute(
                "AllReduce", ALU.add,
                replica_groups=[list(range(NC))],
                ins=[ar_in[:].opt()], outs=[ar_out[:].opt()],
            )
            arB = small.tile([128, NC], f32, tag="arB")
            nc.gpsimd.dma_start(arB[:], _bc(ar_out[:], [[0, 128], [1, NC]]))
            totals = small.tile([128, B], f32, tag="totals")
            nc.vector.reduce_sum(
                out=totals[:], in_=arB[:].rearrange("p (b t) -> p b t", t=2),
                axis=AX.X,
            )
            viol = small.tile([128, B], f32, tag="viol")
            nc.vector.tensor_tensor(
                out=viol[:], in0=totals[:], in1=e4B[:], op=ALU.subtract
            )
            vsum = small.tile([128, 1], f32, tag="vsum")
            nc.vector.tensor_reduce(
                out=vsum[:], in_=viol[:], axis=AX.X, op=ALU.add,
                apply_absolute_value=True,
            )
            flag = small.tile([128, 1], f32, tag="flag")
            nc.vector.tensor_scalar(
                out=flag[:], in0=vsum[:], scalar1=0.1 * B, scalar2=None,
                op0=ALU.is_gt,
            )
            scr4 = small.tile([128, B], f32, tag="scr4")
            tot_own = small.tile([128, 1], f32, tag="tot_own")
            nc.vector.tensor_tensor(out=scr4[:], in0=totals[:], in1=sel4B[:], op=ALU.mult)
            nc.vector.reduce_sum(out=tot_own[:], in_=scr4[:], axis=AX.X)
            scr4b = small.tile([128, B], f32, tag="scr4b")
            e_own = small.tile([128, 1], f32, tag="e_own")
            nc.vector.tensor_tensor(out=scr4b[:], in0=e4B[:], in1=sel4B[:], op=ALU.mult)
            nc.vector.reduce_sum(out=e_own[:], in_=scr4b[:], axis=AX.X)
            nc.vector.tensor_scalar_add(tot_own[:], tot_own[:], 1e-8)
            rec = small.tile([128, 1], f32, tag="rec")
            nc.vector.reciprocal(out=rec[:], in_=tot_own[:])
            fcorr = small.tile([128, 1], f32, tag="fcorr")
            nc.vector.tensor_tensor(
                out=fcorr[:], in0=rec[:], in1=e_own[:], op=ALU.mult
            )
            nc.vector.tensor_scalar(
                out=fcorr[:], in0=fcorr[:], scalar1=1.0, scalar2=None,
                op0=ALU.subtract,
            )
            f_col = small.tile([128, 1], f32, tag="f_col")
            nc.vector.tensor_tensor(
                out=f_col[:], in0=fcorr[:], in1=flag[:], op=ALU.mult
            )
            nc.vector.tensor_scalar_add(f_col[:], f_col[:], 1.0)

            # ---------------- stage D: pass 2 re-softmax + attend ----------------
            with (
                tc.tile_pool(name="ps_s2", bufs=2, space="PSUM") as pss2,
                tc.tile_pool(name="ps_r2", bufs=1, space="PSUM") as psr2,
                tc.tile_pool(name="ps_a", bufs=1, space="PSUM") as psa,
            ):
                for h in range(HL):
                    j, half = h // 2, h % 2
                    dlo = half * DH
                    rrB = work.tile([128, S], f32, tag="w4")
                    nc.gpsimd.dma_start(
                        rrB[:], _bc(RRD[:], [[0, 128], [1, S]], off_elems=h * S)
                    )
                    pr2 = psr2.tile([128, S], f32, tag="pr2")
                    pa = psa.tile([64, S], f32, tag="pa")
                    for kc in range(KC):
                        ps_ = pss2.tile([128, S], f32, tag="ps2")
                        for nh in range(2):
                            nc.tensor.matmul(
                                ps_[:, nh * 512:(nh + 1) * 512],
                                KTs[j][dlo:dlo + DH, kc * 128:(kc + 1) * 128],
                                QTs[j][dlo:dlo + DH, nh * 512:(nh + 1) * 512],
                                start=True, stop=True,
                            )
                        ex = work.tile([128, S], f32, tag="w0")
                        nc.scalar.activation(ex[:], ps_[:], AF.Exp, scale=scale)
                        nc.vector.tensor_mul(ex[:], ex[:], Ps[kc][:])
                        nc.vector.tensor_mul(ex[:], ex[:], rrB[:])
                        u2 = upool.tile([128, S], f32r, tag="u")
                        nc.scalar.activation(
                            u2[:], ex[:], AF.Exp, scale=f_col[:, 0:1]
                        )
                        for nh in range(2):
                            sl = slice(nh * 512, (nh + 1) * 512)
                            nc.tensor.matmul(
                                pr2[:, sl], ones[:], u2[:, sl],
                                start=(kc == 0), stop=(kc == KC - 1),
                            )
                            nc.tensor.matmul(
                                pa[:, sl],
                                Vs[kc][:, h * DH:(h + 1) * DH],
                                u2[:, sl],
                                start=(kc == 0), stop=(kc == KC - 1),
                            )
                    pr2s = work.tile([128, S], f32, tag="w2")
                    nc.scalar.copy(pr2s[:], pr2[:])
                    rr2 = work.tile([128, S], f32, tag="w1")
                    nc.vector.reciprocal_approx_fast(out=rr2[:], in_=pr2s[:])
                    if half == 0:
                        nc.vector.tensor_tensor(
                            out=ATs[j][0:DH, :], in0=pa[:],
                            in1=rr2[0:DH, :], op=ALU.mult,
                        )
                    else:
                        stg = upool.tile([64, S], f32r, tag="stg")
                        nc.vector.tensor_tensor(
                            out=stg[:], in0=pa[:], in1=rr2[0:DH, :], op=ALU.mult,
                        )
                        nc.sync.dma_start(ATs[j][DH:128, :], stg[:])

            # ---------------- stage E: out-proj + RS + layernorm ----------------
            po = dram.tile([S, D], f32, tag="po")
            po_half = dram.tile([QH, D], f32, tag="po_half")
            with (
                tc.tile_pool(name="wo", bufs=1) as wop,
                tc.tile_pool(name="lnc", bufs=1) as lnc,
                tc.tile_pool(name="ps_o", bufs=3, space="PSUM") as pso,
            ):
                boB = lnc.tile([128, D], f32, tag="boB")
                nc.gpsimd.dma_start(boB[:], _bc(BO[:], [[0, 128], [1, D]]))
                gamB = lnc.tile([128, D], f32, tag="gamB")
                nc.gpsimd.dma_start(gamB[:], _bc(GAM[:], [[0, 128], [1, D]]))
                betB = lnc.tile([128, D], f32, tag="betB")
                nc.gpsimd.dma_start(betB[:], _bc(BET[:], [[0, 128], [1, D]]))
                wos = []
                for hc in range(4):
                    wt = wop.tile([128, D], f32r, tag=f"wo{hc}", name=f"wo{hc}")
                    nc.sync.dma_start(wt[:], WOT[hc * 128:(hc + 1) * 128, :])
                    wos.append(wt)
                for qc in range(KC):
                    pp = pso.tile([128, D], f32, tag="po")
                    for nh in range(2):
                        for hc in range(4):
                            nc.tensor.matmul(
                                pp[:, nh * 512:(nh + 1) * 512],
                                ATs[hc][:, qc * 128:(qc + 1) * 128],
                                wos[hc][:, nh * 512:(nh + 1) * 512],
                                start=(hc == 0), stop=(hc == 3),
                            )
                    ob = work.tile([128, D], f32, tag="w5")
                    nc.scalar.copy(ob[:], pp[:])
                    nc.sync.dma_start(po[qc * 128:(qc + 1) * 128, :], ob[:])
                nc.gpsimd.collective_compute(
                    "ReduceScatter", ALU.add,
                    replica_groups=[[0, 1], [2, 3], [4, 5], [6, 7]],
                    ins=[po[:].opt()], outs=[po_half[:].opt()],
                )
                for qc in range(4):
                    rs = work.tile([128, D], f32, tag="w0")
                    nc.sync.dma_start(rs[:], po_half[qc * 128:(qc + 1) * 128, :])
                    qh_t = work.tile([128, D], f32, tag="w1")
                    nc.sync.dma_start(qh_t[:], QHALF[qc * 128:(qc + 1) * 128, :])
                    x = work.tile([128, D], f32, tag="w2")
                    nc.vector.tensor_add(x[:], rs[:], qh_t[:])
                    nc.vector.tensor_add(x[:], x[:], boB[:])
                    stats = small.tile([128, 2, 6], f32, tag="stats")
                    xv = x[:].rearrange("p (s n) -> p s n", s=2)
                    for sgi in range(2):
                        nc.vector.bn_stats(out=stats[:, sgi, :], in_=xv[:, sgi, :])
                    mv = small.tile([128, 2], f32, tag="mv")
                    nc.vector.bn_aggr(out=mv[:], in_=stats[:])
                    sd = small.tile([128, 1], f32, tag="sd")
                    nc.scalar.activation(
                        sd[:], mv[:, 1:2], AF.Sqrt, bias=eps_t[:, 0:1]
                    )
                    rstd = small.tile([128, 1], f32, tag="rstd")
                    nc.vector.reciprocal(out=rstd[:], in_=sd[:])
                    y = work.tile([128, D], f32, tag="w3")
                    nc.vector.tensor_scalar(
                        out=y[:], in0=x[:], scalar1=mv[:, 0:1], scalar2=rstd[:],
                        op0=ALU.subtract, op1=ALU.mult,
                    )
                    nc.vector.tensor_mul(y[:], y[:], gamB[:])
                    nc.vector.tensor_add(y[:], y[:], betB[:])
                    nc.sync.dma_start(OUT[qc * 128:(qc + 1) * 128, :], y[:])

    nc.compile()
    return nc


_NC_CACHE = None


def _get_nc():
    global _NC_CACHE
    if _NC_CACHE is None:
        _NC_CACHE = build_kernel()
    return _NC_CACHE


def make_in_maps(inputs):
    q = np.asarray(inputs["query"], np.float32)
    k = np.asarray(inputs["key"], np.float32)
    v = np.asarray(inputs["value"], np.float32)
    co = np.asarray(inputs["spacetime_coords"], np.float32)
    E = np.asarray(inputs["initial_energy"], np.float32)
    Wq = np.asarray(inputs["Wq"], np.float32)
    Wk = np.asarray(inputs["Wk"], np.float32)
    Wv = np.asarray(inputs["Wv"], np.float32)
    Wo = np.asarray(inputs["Wo"], np.float32)
    bq = np.asarray(inputs["bq"], np.float32)
    bk = np.asarray(inputs["bk"], np.float32)
    bv = np.asarray(inputs["bv"], np.float32)
    bo = np.asarray(inputs["bo"], np.float32)
    We = np.asarray(inputs["We"], np.float32)
    be = np.asarray(inputs["be"], np.float32)
    gam = np.asarray(inputs["gamma"], np.float32)
    bet = np.asarray(inputs["beta"], np.float32)

    maps = []
    for c in range(NC):
        b, g = c // 2, c % 2
        hs = slice(SH * g, SH * (g + 1))
        qlo = QH * g
        sel4 = np.zeros((1, B), np.float32); sel4[0, b] = 1.0
        slot8 = np.zeros((1, NC), np.float32); slot8[0, c] = 1.0
        maps.append({
            "queryT": np.ascontiguousarray(q[b].T),
            "keyT": np.ascontiguousarray(k[b].T),
            "valueT": np.ascontiguousarray(v[b].T),
            "query_half": np.ascontiguousarray(q[b, qlo:qlo + QH]),
            "WqT": np.ascontiguousarray(Wq[hs].T),
            "WkT": np.ascontiguousarray(Wk[hs].T),
            "WvT": np.ascontiguousarray(Wv[hs].T),
            "WoT_own": np.ascontiguousarray(Wo.T[hs]),
            "bq_half": np.ascontiguousarray(bq[hs]),
            "bk_half": np.ascontiguousarray(bk[hs]),
            "bv_half": np.ascontiguousarray(bv[hs]),
            "bo": bo,
            "gamma": gam,
            "beta": bet,
            "coords": np.ascontiguousarray(co[b]),
            "coordsT": np.ascontiguousarray(co[b].T),
            "We_row": We.reshape(1, DH),
            "be": be.reshape(1),
            "E4": E.reshape(1, B),
            "sel4": sel4,
            "slot8": slot8,
        })
    return maps


def kernel(**inputs) -> np.ndarray:
    nc = _get_nc()
    maps = make_in_maps(inputs)
    res = run_bass_kernel_spmd(nc, maps, core_ids=list(range(NC)))
    out = np.empty((B, S, D), np.float32)
    for c in range(NC):
        b, g = c // 2, c % 2
        out[b, QH * g:QH * (g + 1), :] = res.results[c]["out_half"]
    return out

